# revision 24
# baseline (speedup 1.0000x reference)
"""Multi-head attention (B=4, S=2048, D=1024, H=16) on 8 Trainium2 NeuronCores.

Sharding: data-parallel over the 4 batches x tensor-parallel over head halves
(2 groups of 8 heads).  core c -> batch c//2, heads (c%2)*8 .. (c%2)*8+7.
Each core computes a partial output x[b] attention over its 8 heads projected
through its slice of w_out; the host sums the two partials per batch.

Device algorithm per core (all layouts chosen so no tensor is ever transposed
except x, once, via the PE):
  1. xT = x.T  (PE transpose, 128x128 tiles, block-pipelined)
  2. qT = wq.T @ xT + bq ; kT = wk.T @ xT + bk   ([512, 2048], head-major rows)
     v  = xT.T @ wv + bv                          ([2048, 512] natural layout)
     v is augmented with a ones-column per head -> AV matmul also produces the
     softmax denominator for free (M=65).
  3. per (head, s_q half, s_k chunk):  scoresT = kT.T@qT -> PSUM,
     expm = exp(scores/8) (ACT, fp16 out), expm *= mask (DVE fp16 2x),
     ctxT[65, s_q] += v_aug.T @ expm  (PSUM accumulation over chunks).
     ctxT = ctx_unnorm / denom  via DVE reciprocal + gpsimd partition_broadcast.
  4. out_partial = ctxT.T @ wo + bo  (bo only on even cores; host sums pairs).
"""

import os
import sys
import math
from contextlib import ExitStack

import numpy as np

if "/opt/trn_rl_repo" not in sys.path:
    sys.path.insert(0, "/opt/trn_rl_repo")

B, S, D, H = 4, 2048, 1024, 16
DH = 64          # head dim
HPC = 8          # heads per core
CD = HPC * DH    # 512 cols per core per q/k/v
NCORES = 8

SK_CHUNKS = 16       # s_k chunks of 128
NHALF = 2            # s_q halves of 1024
NPAIR = 4            # head pairs per core
NBLK = 4             # s-blocks of 512 for projection phase


def _build():
    import concourse.bass as bass
    import concourse.mybir as mybir
    import concourse.tile as tile
    from concourse import bacc
    from concourse.bass import ds, ts
    from concourse.masks import make_identity

    f32 = mybir.dt.float32
    f16 = mybir.dt.float16
    i32 = mybir.dt.int32
    Alu = mybir.AluOpType
    Act = mybir.ActivationFunctionType

    nc = bacc.Bacc(name="mha8")

    x_d = nc.dram_tensor("x", [S, D], f32, kind="ExternalInput")
    mask_d = nc.dram_tensor("mask", [S, S], i32, kind="ExternalInput")
    wq_d = nc.dram_tensor("wq", [D, CD], f32, kind="ExternalInput")
    wk_d = nc.dram_tensor("wk", [D, CD], f32, kind="ExternalInput")
    wv_d = nc.dram_tensor("wv", [D, CD], f32, kind="ExternalInput")
    bq_d = nc.dram_tensor("bq", [CD], f32, kind="ExternalInput")
    bk_d = nc.dram_tensor("bk", [CD], f32, kind="ExternalInput")
    bv_d = nc.dram_tensor("bv", [CD], f32, kind="ExternalInput")
    wo_d = nc.dram_tensor("wo", [CD, D], f32, kind="ExternalInput")
    bo_d = nc.dram_tensor("bo", [D], f32, kind="ExternalInput")
    out_d = nc.dram_tensor("out", [S, D], f32, kind="ExternalOutput")

    with tile.TileContext(nc) as tc, ExitStack() as top:
        const = top.enter_context(tc.tile_pool(name="const", bufs=1))

        identity = const.tile([128, 128], f16)
        make_identity(nc, identity)

        exp_bias = const.tile([128, 1], f32)
        nc.vector.memset(exp_bias, -4.0)  # exp(s/8 - 4), see EXP_BIAS below

        ones64 = const.tile([128, 64], f16)  # K=1 matmul partition-broadcast
        nc.vector.memset(ones64, 1.0)

        # biases for q/k: [128, 4] -> column m*128+p holds bq[m*128+p]
        bq_sb = const.tile([128, NBLK], f32)
        bk_sb = const.tile([128, NBLK], f32)
        nc.sync.dma_start(out=bq_sb, in_=bq_d.rearrange("(m p) -> p m", p=128))
        nc.sync.dma_start(out=bk_sb, in_=bk_d.rearrange("(m p) -> p m", p=128))

        # persistent activation tensors
        qk_pool = top.enter_context(tc.tile_pool(name="qk", bufs=1))
        qT_sb = qk_pool.tile([128, NPAIR, S], f16)   # q^T, head pair-major
        kT_sb = qk_pool.tile([128, NPAIR, S], f16)
        v_pool = top.enter_context(tc.tile_pool(name="vpool", bufs=1))
        v_sb = v_pool.tile([128, SK_CHUNKS, HPC, 66], f16)  # [.., 0:64]=v, 64=ones
        ctx_pool = top.enter_context(tc.tile_pool(name="ctxp", bufs=1))
        ctxT_sb = ctx_pool.tile([128, NPAIR, S], f16)

        nc.vector.memset(v_sb[:, :, :, 64:65], 1.0)

        # ---------------- phase 1+2: x transpose + QKV projection ----------
        with ExitStack() as ph2:
            wq_sb = None
            wpool = ph2.enter_context(tc.tile_pool(name="wpool", bufs=1))
            wq_sb = wpool.tile([128, 8, CD], f16)
            wk_sb = wpool.tile([128, 8, CD], f16)
            wv_sb = wpool.tile([128, 8, CD], f16)
            # gpsimd DMA casts fp32 -> fp16 in flight
            nc.gpsimd.dma_start(out=wq_sb, in_=wq_d.rearrange("(k p) n -> p k n", p=128))
            nc.gpsimd.dma_start(out=wk_sb, in_=wk_d.rearrange("(k p) n -> p k n", p=128))
            nc.gpsimd.dma_start(out=wv_sb, in_=wv_d.rearrange("(k p) n -> p k n", p=128))

            bv_row = wpool.tile([1, CD], f32)
            nc.sync.dma_start(out=bv_row, in_=bv_d[None, :])
            bv_bc = wpool.tile([128, CD], f32)
            nc.gpsimd.partition_broadcast(bv_bc, bv_row)

            xt_pool = ph2.enter_context(tc.tile_pool(name="xt", bufs=1))
            xstage_pool = ph2.enter_context(tc.tile_pool(name="xstage", bufs=3))
            tr_ps_pool = ph2.enter_context(
                tc.tile_pool(name="trps", bufs=4, space="PSUM")
            )
            prj_ps_pool = ph2.enter_context(
                tc.tile_pool(name="prjps", bufs=4, space="PSUM")
            )

            # transpose all of x first: xT[d%128, k, s]; PE stays dense after
            xT = xt_pool.tile([128, 8, S], f16)
            for sc in range(16):
                xst = xstage_pool.tile([128, D], f16, tag="xst")
                nc.gpsimd.dma_start(out=xst, in_=x_d[ds(sc * 128, 128), :])
                for half in range(2):
                    trp = tr_ps_pool.tile([128, 512], f16, tag="trp")
                    for k4 in range(4):
                        k = half * 4 + k4
                        nc.tensor.transpose(
                            out=trp[:, ds(k4 * 128, 128)],
                            in_=xst[:, ds(k * 128, 128)],
                            identity=identity,
                        )
                    nc.vector.tensor_copy(
                        out=xT[:, ds(half * 4, 4), ds(sc * 128, 128)],
                        in_=trp.rearrange("p (k s) -> p k s", k=4),
                    )

            # qT / kT
            for which, w_sb, b_sb, dst in (
                ("q", wq_sb, bq_sb, qT_sb),
                ("k", wk_sb, bk_sb, kT_sb),
            ):
                for m in range(4):
                    for n in range(NBLK):
                        pps = prj_ps_pool.tile([128, 512], f32, tag="pps")
                        for k in range(8):
                            nc.tensor.matmul(
                                pps,
                                lhsT=w_sb[:, k, ds(m * 128, 128)],
                                rhs=xT[:, k, ds(n * 512, 512)],
                                start=(k == 0),
                                stop=(k == 7),
                            )
                        # col-chunk m = head pair m (partitions 0-63 head 2m,
                        # 64-127 head 2m+1) -> maps directly onto pair layout
                        nc.vector.tensor_scalar_add(
                            out=dst[:, m, ds(n * 512, 512)],
                            in0=pps,
                            scalar1=b_sb[:, ds(m, 1)],
                        )

            # v (natural layout + bias + ones column)
            for m16 in range(16):
                vps = prj_ps_pool.tile([128, 512], f32, tag="pps")
                for k in range(8):
                    nc.tensor.matmul(
                        vps,
                        lhsT=xT[:, k, ds(m16 * 128, 128)],
                        rhs=wv_sb[:, k, :],
                        start=(k == 0),
                        stop=(k == 7),
                    )
                nc.vector.tensor_tensor(
                    out=v_sb[:, m16, :, 0:64],
                    in0=vps.rearrange("p (h e) -> p h e", h=HPC),
                    in1=bv_bc.rearrange("p (h e) -> p h e", h=HPC),
                    op=Alu.add,
                )

        # ---------------- phase 3: attention ------------------------------
        with ExitStack() as ph3:
            mask_pool = ph3.enter_context(tc.tile_pool(name="maskp", bufs=1))
            expm_pool = ph3.enter_context(tc.tile_pool(name="expm", bufs=3))
            sc_ps_pool = ph3.enter_context(
                tc.tile_pool(name="scps", bufs=2, space="PSUM")
            )
            ctx_ps_pool = ph3.enter_context(
                tc.tile_pool(name="ctxps", bufs=2, space="PSUM")
            )
            rc_pool = ph3.enter_context(tc.tile_pool(name="rcp", bufs=1))

            maskf = []
            for hf in range(NHALF):
                mk = mask_pool.tile([128, SK_CHUNKS, 1024], f16, name=f"maskf{hf}")
                maskf.append(mk)
                for j in range(SK_CHUNKS):
                    nc.gpsimd.dma_start(
                        out=mk[:, j, :],
                        in_=mask_d[ds(j * 128, 128), ds(hf * 1024, 1024)],
                    )

            for hf in range(NHALF):
                # head h's denominator lives at partition 32*(h//2), slot h%2
                den_sb = rc_pool.tile([128, 2, 1024], f32, tag="den", name=f"den{hf}")
                nc.vector.memset(den_sb, 1.0)
                ctxU = rc_pool.tile([128, NPAIR, 1024], f16, tag="ctxU", name=f"ctxU{hf}")
                for c in range(NPAIR):
                    ctx_ps = [
                        ctx_ps_pool.tile([128, 1024], f32, tag="ctxps", name=f"ctxps{a}")
                        for a in range(2)
                    ]
                    # software pipeline: AV(j) is emitted after QK(j+1) so the
                    # in-order PE queue never stalls on exp/mask of iter j
                    pending = None

                    def emit_qk(j):
                        scs = [
                            sc_ps_pool.tile(
                                [128, 1024], f32, tag="scps", name=f"scps{j % 2}_{a}"
                            )
                            for a in range(2)
                        ]
                        # interleave a=0/a=1: adjacent MMs hit disjoint PE row
                        # groups (base partitions 0 / 64) and run concurrently
                        for n2 in range(2):
                            for a in range(2):
                                nc.tensor.matmul(
                                    scs[a][:, ds(n2 * 512, 512)],
                                    lhsT=kT_sb[ds(a * 64, 64), c, ds(j * 128, 128)],
                                    rhs=qT_sb[
                                        ds(a * 64, 64),
                                        c,
                                        ds(hf * 1024 + n2 * 512, 512),
                                    ],
                                    start=True,
                                    stop=True,
                                )
                        return scs

                    def emit_mask_av(j, scs):
                        for a in range(2):
                            expm = expm_pool.tile([128, 1024], f16, tag="expm")
                            nc.scalar.activation(
                                out=expm,
                                in_=scs[a],
                                func=Act.Exp,
                                scale=1.0 / math.sqrt(DH),
                                bias=exp_bias,
                            )
                            expm2 = expm_pool.tile([128, 1024], f16, tag="expm2")
                            nc.vector.tensor_tensor(
                                out=expm2,
                                in0=expm,
                                in1=maskf[hf][:, j, :],
                                op=Alu.mult,
                            )
                            for n2 in range(2):
                                nc.tensor.matmul(
                                    ctx_ps[a][0:65, ds(n2 * 512, 512)],
                                    lhsT=v_sb[:, j, c * 2 + a, 0:65],
                                    rhs=expm2[:, ds(n2 * 512, 512)],
                                    start=(j == 0),
                                    stop=(j == SK_CHUNKS - 1),
                                )

                    for j in range(SK_CHUNKS):
                        scs = emit_qk(j)
                        if pending is not None:
                            emit_mask_av(*pending)
                        pending = (j, scs)
                    emit_mask_av(*pending)
                    for a in range(2):
                        # denom row -> batch tile (ACT), unnormalized ctx -> fp16
                        h = c * 2 + a
                        nc.scalar.copy(
                            out=den_sb[ds(32 * (h // 2), 1), h % 2, :],
                            in_=ctx_ps[a][64:65, :],
                        )
                        nc.vector.tensor_copy(
                            out=ctxU[ds(a * 64, 64), c, :], in_=ctx_ps[a][0:64, :]
                        )
                # batched reciprocal of all 8 denominators, then normalize
                den_rec = rc_pool.tile([128, 2, 1024], f32, tag="denr", name=f"denr{hf}")
                nc.vector.reciprocal(den_rec, den_sb)
                den_rec16 = rc_pool.tile(
                    [128, 2, 1024], f16, tag="denr16", name=f"denr16{hf}"
                )
                nc.vector.tensor_copy(out=den_rec16, in_=den_rec)
                for c in range(NPAIR):
                    # broadcast each head's reciprocal across 64 partitions via
                    # a K=1 ones-matmul (partition_broadcast mis-reads nonzero
                    # base partitions on HW), then multiply from PSUM.
                    for a in range(2):
                        h = c * 2 + a
                        g = h // 2
                        rbc_ps = sc_ps_pool.tile(
                            [128, 1024], f32, tag="scps", name=f"rbcps{c}_{a}"
                        )
                        for n2 in range(2):
                            nc.tensor.matmul(
                                rbc_ps[0:64, ds(n2 * 512, 512)],
                                lhsT=ones64[ds(32 * g, 1), :],
                                rhs=den_rec16[ds(32 * g, 1), h % 2, ds(n2 * 512, 512)],
                                start=True,
                                stop=True,
                                tile_position=(32 * g, 0),
                            )
                        nc.vector.tensor_tensor(
                            out=ctxT_sb[ds(a * 64, 64), c, ds(hf * 1024, 1024)],
                            in0=ctxU[ds(a * 64, 64), c, :],
                            in1=rbc_ps[0:64, :],
                            op=Alu.mult,
                        )

        # ---------------- phase 4: output projection -----------------------
        with ExitStack() as ph4:
            out_ps_pool = ph4.enter_context(
                tc.tile_pool(name="outps", bufs=4, space="PSUM")
            )
            ost_pool = ph4.enter_context(tc.tile_pool(name="ost", bufs=3))
            ph4_const = ph4.enter_context(tc.tile_pool(name="ph4c", bufs=1))

            # out-projection weights, cast to fp16: [128, 4, 1024]
            wo_sb = ph4_const.tile([128, 4, D], f16)
            nc.gpsimd.dma_start(
                out=wo_sb, in_=wo_d.rearrange("(r p) n -> p r n", p=128)
            )
            bo_row = ph4_const.tile([1, D], f32)
            nc.sync.dma_start(out=bo_row, in_=bo_d[None, :])
            bo_bc = ph4_const.tile([128, D], f32)
            nc.gpsimd.partition_broadcast(bo_bc, bo_row)
            for m in range(16):
                ops = out_ps_pool.tile([128, D], f32, tag="ops")
                for r in range(4):
                    for n2 in range(2):
                        nc.tensor.matmul(
                            ops[:, ds(n2 * 512, 512)],
                            lhsT=ctxT_sb[:, r, ds(m * 128, 128)],
                            rhs=wo_sb[:, r, ds(n2 * 512, 512)],
                            start=(r == 0),
                            stop=(r == 3),
                        )
                ost = ost_pool.tile([128, D], f32, tag="ost")
                nc.vector.tensor_tensor(out=ost, in0=ops, in1=bo_bc, op=Alu.add)
                nc.sync.dma_start(out=out_d[ds(m * 128, 128), :], in_=ost)

    nc.compile()
    return nc


_NC = None


def _get_nc():
    global _NC
    if _NC is None:
        _NC = _build()
    return _NC


def make_in_maps(inputs):
    x = np.ascontiguousarray(np.asarray(inputs["x"], dtype=np.float32))
    mask = np.ascontiguousarray(np.asarray(inputs["mask"], dtype=np.int32))
    w_qkv = np.asarray(inputs["w_qkv"], dtype=np.float32)
    b_qkv = np.asarray(inputs["b_qkv"], dtype=np.float32)
    w_out = np.asarray(inputs["w_out"], dtype=np.float32)
    b_out = np.asarray(inputs["b_out"], dtype=np.float32)

    in_maps = []
    for core in range(NCORES):
        b = core // 2
        h0 = (core % 2) * CD
        in_maps.append(
            {
                "x": np.ascontiguousarray(x[b]),
                # device wants mask[s_k, s_q] (scores are computed transposed);
                # DRAM holds mask[s_q, s_k] -> transpose during host-side sharding
                "mask": np.ascontiguousarray(mask[b, 0].T),
                "wq": np.ascontiguousarray(w_qkv[:, h0 : h0 + CD]),
                "wk": np.ascontiguousarray(w_qkv[:, D + h0 : D + h0 + CD]),
                "wv": np.ascontiguousarray(w_qkv[:, 2 * D + h0 : 2 * D + h0 + CD]),
                "bq": np.ascontiguousarray(b_qkv[h0 : h0 + CD]),
                "bk": np.ascontiguousarray(b_qkv[D + h0 : D + h0 + CD]),
                "bv": np.ascontiguousarray(b_qkv[2 * D + h0 : 2 * D + h0 + CD]),
                "wo": np.ascontiguousarray(w_out[h0 : h0 + CD, :]),
                "bo": b_out if core % 2 == 0 else np.zeros_like(b_out),
            }
        )
    return in_maps


def gather_out(core_outs):
    return np.stack(
        [core_outs[2 * b] + core_outs[2 * b + 1] for b in range(B)], axis=0
    )


def run(inputs, trace=False):
    """Returns (output, BassKernelResults)."""
    from concourse import bass_utils

    nc = _get_nc()
    in_maps = make_in_maps(inputs)
    res = bass_utils.run_bass_kernel_spmd(
        nc, in_maps, core_ids=list(range(NCORES)), trace=trace
    )
    out = gather_out([r["out"] for r in res.results])
    return out, res


def kernel(**inputs) -> np.ndarray:
    out, _ = run(inputs, trace=False)
    return out


# revision 26
# speedup vs baseline: 1.4142x; 1.4142x over previous
"""Multi-head attention (B=4, S=2048, D=1024, H=16) on 8 Trainium2 NeuronCores.

Sharding: data-parallel over the 4 batches x tensor-parallel over head halves
(2 groups of 8 heads).  core c -> batch c//2, heads (c%2)*8 .. (c%2)*8+7.
Each core computes a partial output x[b] attention over its 8 heads projected
through its slice of w_out; the host sums the two partials per batch.

Device algorithm per core (all layouts chosen so no tensor is ever transposed
except x, once, via the PE):
  1. xT = x.T  (PE transpose, 128x128 tiles, block-pipelined)
  2. qT = wq.T @ xT + bq ; kT = wk.T @ xT + bk   ([512, 2048], head-major rows)
     v  = xT.T @ wv + bv                          ([2048, 512] natural layout)
     v is augmented with a ones-column per head -> AV matmul also produces the
     softmax denominator for free (M=65).
  3. per (head, s_q half, s_k chunk):  scoresT = kT.T@qT -> PSUM,
     expm = exp(scores/8) (ACT, fp16 out), expm *= mask (DVE fp16 2x),
     ctxT[65, s_q] += v_aug.T @ expm  (PSUM accumulation over chunks).
     ctxT = ctx_unnorm / denom  via DVE reciprocal + gpsimd partition_broadcast.
  4. out_partial = ctxT.T @ wo + bo  (bo only on even cores; host sums pairs).
"""

import os
import sys
import math
from contextlib import ExitStack

import numpy as np

if "/opt/trn_rl_repo" not in sys.path:
    sys.path.insert(0, "/opt/trn_rl_repo")

B, S, D, H = 4, 2048, 1024, 16
DH = 64          # head dim
HPC = 8          # heads per core
CD = HPC * DH    # 512 cols per core per q/k/v
NCORES = 8

SK_CHUNKS = 16       # s_k chunks of 128
NHALF = 2            # s_q halves of 1024
NPAIR = 4            # head pairs per core
NBLK = 4             # s-blocks of 512 for projection phase


def _build():
    import concourse.bass as bass
    import concourse.mybir as mybir
    import concourse.tile as tile
    from concourse import bacc
    from concourse.bass import ds, ts
    from concourse.masks import make_identity

    f32 = mybir.dt.float32
    f16 = mybir.dt.float16
    i32 = mybir.dt.int32
    Alu = mybir.AluOpType
    Act = mybir.ActivationFunctionType

    nc = bacc.Bacc(name="mha8")

    x_d = nc.dram_tensor("x", [S, D], f32, kind="ExternalInput")
    mask_d = nc.dram_tensor("mask", [S, S], i32, kind="ExternalInput")
    wq_d = nc.dram_tensor("wq", [D, CD], f32, kind="ExternalInput")
    wk_d = nc.dram_tensor("wk", [D, CD], f32, kind="ExternalInput")
    wv_d = nc.dram_tensor("wv", [D, CD], f32, kind="ExternalInput")
    bq_d = nc.dram_tensor("bq", [CD], f32, kind="ExternalInput")
    bk_d = nc.dram_tensor("bk", [CD], f32, kind="ExternalInput")
    bv_d = nc.dram_tensor("bv", [CD], f32, kind="ExternalInput")
    wo_d = nc.dram_tensor("wo", [CD, D], f32, kind="ExternalInput")
    bo_d = nc.dram_tensor("bo", [D], f32, kind="ExternalInput")
    out_d = nc.dram_tensor("out", [S, D], f32, kind="ExternalOutput")

    with tile.TileContext(nc) as tc, ExitStack() as top:
        const = top.enter_context(tc.tile_pool(name="const", bufs=1))

        identity = const.tile([128, 128], f16)
        make_identity(nc, identity)

        exp_bias = const.tile([128, 1], f32)
        nc.vector.memset(exp_bias, -4.0)  # exp(s/8 - 4), see EXP_BIAS below

        ones64 = const.tile([128, 64], f16)  # K=1 matmul partition-broadcast
        nc.vector.memset(ones64, 1.0)

        # biases for q/k: [128, 4] -> column m*128+p holds bq[m*128+p]
        bq_sb = const.tile([128, NBLK], f32)
        bk_sb = const.tile([128, NBLK], f32)
        nc.sync.dma_start(out=bq_sb, in_=bq_d.rearrange("(m p) -> p m", p=128))
        nc.sync.dma_start(out=bk_sb, in_=bk_d.rearrange("(m p) -> p m", p=128))

        # persistent activation tensors
        qk_pool = top.enter_context(tc.tile_pool(name="qk", bufs=1))
        qT_sb = qk_pool.tile([128, NPAIR, S], f16)   # q^T, head pair-major
        kT_sb = qk_pool.tile([128, NPAIR, S], f16)
        v_pool = top.enter_context(tc.tile_pool(name="vpool", bufs=1))
        v_sb = v_pool.tile([128, SK_CHUNKS, HPC, 66], f16)  # [.., 0:64]=v, 64=ones
        ctx_pool = top.enter_context(tc.tile_pool(name="ctxp", bufs=1))
        ctxT_sb = ctx_pool.tile([128, NPAIR, S], f16)

        nc.vector.memset(v_sb[:, :, :, 64:65], 1.0)

        # ---------------- phase 1+2: x transpose + QKV projection ----------
        with ExitStack() as ph2:
            wq_sb = None
            wpool = ph2.enter_context(tc.tile_pool(name="wpool", bufs=1))
            wq_sb = wpool.tile([128, 8, CD], f16)
            wk_sb = wpool.tile([128, 8, CD], f16)
            wv_sb = wpool.tile([128, 8, CD], f16)
            # gpsimd DMA casts fp32 -> fp16 in flight
            nc.gpsimd.dma_start(out=wq_sb, in_=wq_d.rearrange("(k p) n -> p k n", p=128))
            nc.gpsimd.dma_start(out=wk_sb, in_=wk_d.rearrange("(k p) n -> p k n", p=128))
            nc.gpsimd.dma_start(out=wv_sb, in_=wv_d.rearrange("(k p) n -> p k n", p=128))

            bv_row = wpool.tile([1, CD], f32)
            nc.sync.dma_start(out=bv_row, in_=bv_d[None, :])
            bv_bc = wpool.tile([128, CD], f32)
            nc.gpsimd.partition_broadcast(bv_bc, bv_row)

            xt_pool = ph2.enter_context(tc.tile_pool(name="xt", bufs=1))
            xstage_pool = ph2.enter_context(tc.tile_pool(name="xstage", bufs=3))
            tr_ps_pool = ph2.enter_context(
                tc.tile_pool(name="trps", bufs=4, space="PSUM")
            )
            prj_ps_pool = ph2.enter_context(
                tc.tile_pool(name="prjps", bufs=4, space="PSUM")
            )

            # transpose all of x first: xT[d%128, k, s]; PE stays dense after
            xT = xt_pool.tile([128, 8, S], f16)
            for sc in range(16):
                xst = xstage_pool.tile([128, D], f16, tag="xst")
                nc.gpsimd.dma_start(out=xst, in_=x_d[ds(sc * 128, 128), :])
                for half in range(2):
                    trp = tr_ps_pool.tile([128, 512], f16, tag="trp")
                    for k4 in range(4):
                        k = half * 4 + k4
                        nc.tensor.transpose(
                            out=trp[:, ds(k4 * 128, 128)],
                            in_=xst[:, ds(k * 128, 128)],
                            identity=identity,
                        )
                    nc.vector.tensor_copy(
                        out=xT[:, ds(half * 4, 4), ds(sc * 128, 128)],
                        in_=trp.rearrange("p (k s) -> p k s", k=4),
                    )

            # qT / kT
            for which, w_sb, b_sb, dst in (
                ("q", wq_sb, bq_sb, qT_sb),
                ("k", wk_sb, bk_sb, kT_sb),
            ):
                for m in range(4):
                    for n in range(NBLK):
                        pps = prj_ps_pool.tile([128, 512], f32, tag="pps")
                        for k in range(8):
                            nc.tensor.matmul(
                                pps,
                                lhsT=w_sb[:, k, ds(m * 128, 128)],
                                rhs=xT[:, k, ds(n * 512, 512)],
                                start=(k == 0),
                                stop=(k == 7),
                            )
                        # col-chunk m = head pair m (partitions 0-63 head 2m,
                        # 64-127 head 2m+1) -> maps directly onto pair layout
                        nc.vector.tensor_scalar_add(
                            out=dst[:, m, ds(n * 512, 512)],
                            in0=pps,
                            scalar1=b_sb[:, ds(m, 1)],
                        )

            # v (natural layout + bias + ones column)
            for m16 in range(16):
                vps = prj_ps_pool.tile([128, 512], f32, tag="pps")
                for k in range(8):
                    nc.tensor.matmul(
                        vps,
                        lhsT=xT[:, k, ds(m16 * 128, 128)],
                        rhs=wv_sb[:, k, :],
                        start=(k == 0),
                        stop=(k == 7),
                    )
                nc.vector.tensor_tensor(
                    out=v_sb[:, m16, :, 0:64],
                    in0=vps.rearrange("p (h e) -> p h e", h=HPC),
                    in1=bv_bc.rearrange("p (h e) -> p h e", h=HPC),
                    op=Alu.add,
                )

        # ---------------- phase 3: attention ------------------------------
        with ExitStack() as ph3:
            mask_pool = ph3.enter_context(tc.tile_pool(name="maskp", bufs=1))
            expm_pool = ph3.enter_context(tc.tile_pool(name="expm", bufs=3))
            sc_ps_pool = ph3.enter_context(
                tc.tile_pool(name="scps", bufs=1, space="PSUM")
            )
            ctx_ps_pool = ph3.enter_context(
                tc.tile_pool(name="ctxps", bufs=2, space="PSUM")
            )
            rc_pool = ph3.enter_context(tc.tile_pool(name="rcp", bufs=1))

            maskf = []
            for hf in range(NHALF):
                mk = mask_pool.tile([128, SK_CHUNKS, 1024], f16, name=f"maskf{hf}")
                maskf.append(mk)
                for j in range(SK_CHUNKS):
                    nc.gpsimd.dma_start(
                        out=mk[:, j, :],
                        in_=mask_d[ds(j * 128, 128), ds(hf * 1024, 1024)],
                    )

            for hf in range(NHALF):
                # head h's denominator lives at partition 32*(h//2), slot h%2
                den_sb = rc_pool.tile([128, 2, 1024], f32, tag="den", name=f"den{hf}")
                nc.vector.memset(den_sb, 1.0)
                ctxU = rc_pool.tile([128, NPAIR, 1024], f16, tag="ctxU", name=f"ctxU{hf}")
                for c in range(NPAIR):
                    ctx_ps = [
                        ctx_ps_pool.tile([128, 1024], f32, tag="ctxps", name=f"ctxps{a}")
                        for a in range(2)
                    ]
                    # software pipeline: AV(j) is emitted after QK(j+1) so the
                    # in-order PE queue never stalls on exp/mask of iter j
                    pending = None

                    def emit_qk(j):
                        sc = sc_ps_pool.tile(
                            [128, 2, 1024], f32, tag="scps", name=f"scps{j % 2}"
                        )
                        # interleave a=0/a=1: adjacent MMs hit disjoint PE row
                        # groups (base partitions 0 / 64) and run concurrently
                        for n2 in range(2):
                            for a in range(2):
                                nc.tensor.matmul(
                                    sc[:, a, ds(n2 * 512, 512)],
                                    lhsT=kT_sb[ds(a * 64, 64), c, ds(j * 128, 128)],
                                    rhs=qT_sb[
                                        ds(a * 64, 64),
                                        c,
                                        ds(hf * 1024 + n2 * 512, 512),
                                    ],
                                    start=True,
                                    stop=True,
                                )
                        return sc

                    def emit_mask_av(j, sc):
                        # one big exp + one big mask multiply for both heads
                        expm = expm_pool.tile([128, 2, 1024], f16, tag="expm")
                        nc.scalar.activation(
                            out=expm,
                            in_=sc,
                            func=Act.Exp,
                            scale=1.0 / math.sqrt(DH),
                            bias=exp_bias,
                        )
                        expm2 = expm_pool.tile([128, 2, 1024], f16, tag="expm2")
                        nc.vector.tensor_tensor(
                            out=expm2,
                            in0=expm,
                            in1=maskf[hf][:, j, None, :].to_broadcast((128, 2, 1024)),
                            op=Alu.mult,
                        )
                        for a in range(2):
                            for n2 in range(2):
                                nc.tensor.matmul(
                                    ctx_ps[a][0:65, ds(n2 * 512, 512)],
                                    lhsT=v_sb[:, j, c * 2 + a, 0:65],
                                    rhs=expm2[:, a, ds(n2 * 512, 512)],
                                    start=(j == 0),
                                    stop=(j == SK_CHUNKS - 1),
                                )

                    for j in range(SK_CHUNKS):
                        scs = emit_qk(j)
                        if pending is not None:
                            emit_mask_av(*pending)
                        pending = (j, scs)
                    emit_mask_av(*pending)
                    for a in range(2):
                        # denom row -> batch tile (ACT), unnormalized ctx -> fp16
                        h = c * 2 + a
                        nc.scalar.copy(
                            out=den_sb[ds(32 * (h // 2), 1), h % 2, :],
                            in_=ctx_ps[a][64:65, :],
                        )
                        nc.vector.tensor_copy(
                            out=ctxU[ds(a * 64, 64), c, :], in_=ctx_ps[a][0:64, :]
                        )
                # batched reciprocal of all 8 denominators, then normalize
                den_rec = rc_pool.tile([128, 2, 1024], f32, tag="denr", name=f"denr{hf}")
                nc.vector.reciprocal(den_rec, den_sb)
                den_rec16 = rc_pool.tile(
                    [128, 2, 1024], f16, tag="denr16", name=f"denr16{hf}"
                )
                nc.vector.tensor_copy(out=den_rec16, in_=den_rec)
                for c in range(NPAIR):
                    # broadcast each head's reciprocal across 64 partitions via
                    # a K=1 ones-matmul (partition_broadcast mis-reads nonzero
                    # base partitions on HW), then multiply from PSUM.
                    for a in range(2):
                        h = c * 2 + a
                        g = h // 2
                        rbc_ps = sc_ps_pool.tile(
                            [128, 1024], f32, tag="scps", name=f"rbcps{c}_{a}"
                        )
                        for n2 in range(2):
                            nc.tensor.matmul(
                                rbc_ps[0:64, ds(n2 * 512, 512)],
                                lhsT=ones64[ds(32 * g, 1), :],
                                rhs=den_rec16[ds(32 * g, 1), h % 2, ds(n2 * 512, 512)],
                                start=True,
                                stop=True,
                                tile_position=(32 * g, 0),
                            )
                        nc.vector.tensor_tensor(
                            out=ctxT_sb[ds(a * 64, 64), c, ds(hf * 1024, 1024)],
                            in0=ctxU[ds(a * 64, 64), c, :],
                            in1=rbc_ps[0:64, :],
                            op=Alu.mult,
                        )

        # ---------------- phase 4: output projection -----------------------
        with ExitStack() as ph4:
            out_ps_pool = ph4.enter_context(
                tc.tile_pool(name="outps", bufs=4, space="PSUM")
            )
            ost_pool = ph4.enter_context(tc.tile_pool(name="ost", bufs=3))
            ph4_const = ph4.enter_context(tc.tile_pool(name="ph4c", bufs=1))

            # out-projection weights, cast to fp16: [128, 4, 1024]
            wo_sb = ph4_const.tile([128, 4, D], f16)
            nc.gpsimd.dma_start(
                out=wo_sb, in_=wo_d.rearrange("(r p) n -> p r n", p=128)
            )
            bo_row = ph4_const.tile([1, D], f32)
            nc.sync.dma_start(out=bo_row, in_=bo_d[None, :])
            bo_bc = ph4_const.tile([128, D], f32)
            nc.gpsimd.partition_broadcast(bo_bc, bo_row)
            for m in range(16):
                ops = out_ps_pool.tile([128, D], f32, tag="ops")
                for r in range(4):
                    for n2 in range(2):
                        nc.tensor.matmul(
                            ops[:, ds(n2 * 512, 512)],
                            lhsT=ctxT_sb[:, r, ds(m * 128, 128)],
                            rhs=wo_sb[:, r, ds(n2 * 512, 512)],
                            start=(r == 0),
                            stop=(r == 3),
                        )
                ost = ost_pool.tile([128, D], f32, tag="ost")
                nc.vector.tensor_tensor(out=ost, in0=ops, in1=bo_bc, op=Alu.add)
                nc.sync.dma_start(out=out_d[ds(m * 128, 128), :], in_=ost)

    nc.compile()
    return nc


_NC = None


def _get_nc():
    global _NC
    if _NC is None:
        _NC = _build()
    return _NC


def make_in_maps(inputs):
    x = np.ascontiguousarray(np.asarray(inputs["x"], dtype=np.float32))
    mask = np.ascontiguousarray(np.asarray(inputs["mask"], dtype=np.int32))
    w_qkv = np.asarray(inputs["w_qkv"], dtype=np.float32)
    b_qkv = np.asarray(inputs["b_qkv"], dtype=np.float32)
    w_out = np.asarray(inputs["w_out"], dtype=np.float32)
    b_out = np.asarray(inputs["b_out"], dtype=np.float32)

    in_maps = []
    for core in range(NCORES):
        b = core // 2
        h0 = (core % 2) * CD
        in_maps.append(
            {
                "x": np.ascontiguousarray(x[b]),
                # device wants mask[s_k, s_q] (scores are computed transposed);
                # DRAM holds mask[s_q, s_k] -> transpose during host-side sharding
                "mask": np.ascontiguousarray(mask[b, 0].T),
                "wq": np.ascontiguousarray(w_qkv[:, h0 : h0 + CD]),
                "wk": np.ascontiguousarray(w_qkv[:, D + h0 : D + h0 + CD]),
                "wv": np.ascontiguousarray(w_qkv[:, 2 * D + h0 : 2 * D + h0 + CD]),
                "bq": np.ascontiguousarray(b_qkv[h0 : h0 + CD]),
                "bk": np.ascontiguousarray(b_qkv[D + h0 : D + h0 + CD]),
                "bv": np.ascontiguousarray(b_qkv[2 * D + h0 : 2 * D + h0 + CD]),
                "wo": np.ascontiguousarray(w_out[h0 : h0 + CD, :]),
                "bo": b_out if core % 2 == 0 else np.zeros_like(b_out),
            }
        )
    return in_maps


def gather_out(core_outs):
    return np.stack(
        [core_outs[2 * b] + core_outs[2 * b + 1] for b in range(B)], axis=0
    )


def run(inputs, trace=False):
    """Returns (output, BassKernelResults)."""
    from concourse import bass_utils

    nc = _get_nc()
    in_maps = make_in_maps(inputs)
    res = bass_utils.run_bass_kernel_spmd(
        nc, in_maps, core_ids=list(range(NCORES)), trace=trace
    )
    out = gather_out([r["out"] for r in res.results])
    return out, res


def kernel(**inputs) -> np.ndarray:
    out, _ = run(inputs, trace=False)
    return out


# revision 29
# speedup vs baseline: 1.5065x; 1.0652x over previous
"""Multi-head attention (B=4, S=2048, D=1024, H=16) on 8 Trainium2 NeuronCores.

Sharding: data-parallel over the 4 batches x tensor-parallel over head halves
(2 groups of 8 heads).  core c -> batch c//2, heads (c%2)*8 .. (c%2)*8+7.
Each core computes a partial output x[b] attention over its 8 heads projected
through its slice of w_out; the host sums the two partials per batch.

Device algorithm per core (all layouts chosen so no tensor is ever transposed
except x, once, via the PE):
  1. xT = x.T  (PE transpose, 128x128 tiles, block-pipelined)
  2. qT = wq.T @ xT + bq ; kT = wk.T @ xT + bk   ([512, 2048], head-major rows)
     v  = xT.T @ wv + bv                          ([2048, 512] natural layout)
     v is augmented with a ones-column per head -> AV matmul also produces the
     softmax denominator for free (M=65).
  3. per (head, s_q half, s_k chunk):  scoresT = kT.T@qT -> PSUM,
     expm = exp(scores/8) (ACT, fp16 out), expm *= mask (DVE fp16 2x),
     ctxT[65, s_q] += v_aug.T @ expm  (PSUM accumulation over chunks).
     ctxT = ctx_unnorm / denom  via DVE reciprocal + gpsimd partition_broadcast.
  4. out_partial = ctxT.T @ wo + bo  (bo only on even cores; host sums pairs).
"""

import os
import sys
import math
from contextlib import ExitStack

import numpy as np

if "/opt/trn_rl_repo" not in sys.path:
    sys.path.insert(0, "/opt/trn_rl_repo")

B, S, D, H = 4, 2048, 1024, 16
DH = 64          # head dim
HPC = 8          # heads per core
CD = HPC * DH    # 512 cols per core per q/k/v
NCORES = 8

SK_CHUNKS = 16       # s_k chunks of 128
NHALF = 2            # s_q halves of 1024
NPAIR = 4            # head pairs per core
NBLK = 4             # s-blocks of 512 for projection phase


def _build():
    import concourse.bass as bass
    import concourse.mybir as mybir
    import concourse.tile as tile
    from concourse import bacc
    from concourse.bass import ds, ts
    from concourse.masks import make_identity

    f32 = mybir.dt.float32
    f16 = mybir.dt.float16
    i32 = mybir.dt.int32
    Alu = mybir.AluOpType
    Act = mybir.ActivationFunctionType

    nc = bacc.Bacc(name="mha8")

    x_d = nc.dram_tensor("x", [S, D], f32, kind="ExternalInput")
    mask_d = nc.dram_tensor("mask", [S, S], i32, kind="ExternalInput")
    wq_d = nc.dram_tensor("wq", [D, CD], f32, kind="ExternalInput")
    wk_d = nc.dram_tensor("wk", [D, CD], f32, kind="ExternalInput")
    wv_d = nc.dram_tensor("wv", [D, CD], f32, kind="ExternalInput")
    bq_d = nc.dram_tensor("bq", [CD], f32, kind="ExternalInput")
    bk_d = nc.dram_tensor("bk", [CD], f32, kind="ExternalInput")
    bv_d = nc.dram_tensor("bv", [CD], f32, kind="ExternalInput")
    wo_d = nc.dram_tensor("wo", [CD, D], f32, kind="ExternalInput")
    bo_d = nc.dram_tensor("bo", [D], f32, kind="ExternalInput")
    out_d = nc.dram_tensor("out", [S, D], f32, kind="ExternalOutput")

    with tile.TileContext(nc) as tc, ExitStack() as top:
        const = top.enter_context(tc.tile_pool(name="const", bufs=1))

        identity = const.tile([128, 128], f16)
        make_identity(nc, identity)

        exp_bias = const.tile([128, 1], f32)
        nc.vector.memset(exp_bias, -4.0)  # exp(s/8 - 4), see EXP_BIAS below

        ones64 = const.tile([128, 64], f16)  # K=1 matmul partition-broadcast
        nc.vector.memset(ones64, 1.0)

        # biases for q/k: [128, 4] -> column m*128+p holds bq[m*128+p]
        bq_sb = const.tile([128, NBLK], f32)
        bk_sb = const.tile([128, NBLK], f32)
        nc.sync.dma_start(out=bq_sb, in_=bq_d.rearrange("(m p) -> p m", p=128))
        nc.sync.dma_start(out=bk_sb, in_=bk_d.rearrange("(m p) -> p m", p=128))

        # persistent activation tensors
        qk_pool = top.enter_context(tc.tile_pool(name="qk", bufs=1))
        qT_sb = qk_pool.tile([128, NPAIR, S], f16)   # q^T, head pair-major
        kT_sb = qk_pool.tile([128, NPAIR, S], f16)
        v_pool = top.enter_context(tc.tile_pool(name="vpool", bufs=1))
        v_sb = v_pool.tile([128, SK_CHUNKS, HPC, 66], f16)  # [.., 0:64]=v, 64=ones
        ctx_pool = top.enter_context(tc.tile_pool(name="ctxp", bufs=1))
        ctxT_sb = ctx_pool.tile([128, NPAIR, S], f16)

        nc.vector.memset(v_sb[:, :, :, 64:65], 1.0)

        # ---------------- phase 1+2: x transpose + QKV projection ----------
        with ExitStack() as ph2:
            wpool = ph2.enter_context(tc.tile_pool(name="wpool", bufs=1))
            xt_pool = ph2.enter_context(tc.tile_pool(name="xt", bufs=1))
            # all 16 stage slots at once: the up-front x DMAs must not block
            # the SWDGE queue (weight loads sit behind them)
            xstage_pool = ph2.enter_context(tc.tile_pool(name="xstage", bufs=16))
            tr_ps_pool = ph2.enter_context(
                tc.tile_pool(name="trps", bufs=4, space="PSUM")
            )
            prj_ps_pool = ph2.enter_context(
                tc.tile_pool(name="prjps", bufs=4, space="PSUM")
            )

            # x loads go first -- the PE's transpose work is gated on them;
            # the 12MB of weight loads would otherwise queue ahead on SWDGE
            xsts = []
            for sc in range(16):
                xst = xstage_pool.tile([128, D], f16, tag="xst", name=f"xst{sc}")
                nc.gpsimd.dma_start(out=xst, in_=x_d[ds(sc * 128, 128), :])
                xsts.append(xst)

            wq_sb = wpool.tile([128, 8, CD], f16)
            wk_sb = wpool.tile([128, 8, CD], f16)
            wv_sb = wpool.tile([128, 8, CD], f16)
            # gpsimd DMA casts fp32 -> fp16 in flight
            nc.gpsimd.dma_start(out=wq_sb, in_=wq_d.rearrange("(k p) n -> p k n", p=128))
            nc.gpsimd.dma_start(out=wk_sb, in_=wk_d.rearrange("(k p) n -> p k n", p=128))
            nc.gpsimd.dma_start(out=wv_sb, in_=wv_d.rearrange("(k p) n -> p k n", p=128))

            bv_row = wpool.tile([1, CD], f32)
            nc.sync.dma_start(out=bv_row, in_=bv_d[None, :])
            bv_bc = wpool.tile([128, CD], f32)
            nc.gpsimd.partition_broadcast(bv_bc, bv_row)

            # transpose all of x: xT[d%128, k, s]; PE stays dense after
            xT = xt_pool.tile([128, 8, S], f16)
            for sc in range(16):
                xst = xsts[sc]
                for half in range(2):
                    trp = tr_ps_pool.tile([128, 512], f16, tag="trp")
                    for k4 in range(4):
                        k = half * 4 + k4
                        nc.tensor.transpose(
                            out=trp[:, ds(k4 * 128, 128)],
                            in_=xst[:, ds(k * 128, 128)],
                            identity=identity,
                        )
                    nc.vector.tensor_copy(
                        out=xT[:, ds(half * 4, 4), ds(sc * 128, 128)],
                        in_=trp.rearrange("p (k s) -> p k s", k=4),
                    )

            # qT / kT
            for which, w_sb, b_sb, dst in (
                ("q", wq_sb, bq_sb, qT_sb),
                ("k", wk_sb, bk_sb, kT_sb),
            ):
                for m in range(4):
                    for n in range(NBLK):
                        pps = prj_ps_pool.tile([128, 512], f32, tag="pps")
                        for k in range(8):
                            nc.tensor.matmul(
                                pps,
                                lhsT=w_sb[:, k, ds(m * 128, 128)],
                                rhs=xT[:, k, ds(n * 512, 512)],
                                start=(k == 0),
                                stop=(k == 7),
                            )
                        # col-chunk m = head pair m (partitions 0-63 head 2m,
                        # 64-127 head 2m+1) -> maps directly onto pair layout
                        nc.vector.tensor_scalar_add(
                            out=dst[:, m, ds(n * 512, 512)],
                            in0=pps,
                            scalar1=b_sb[:, ds(m, 1)],
                        )

            # v (natural layout + bias + ones column)
            for m16 in range(16):
                vps = prj_ps_pool.tile([128, 512], f32, tag="pps")
                for k in range(8):
                    nc.tensor.matmul(
                        vps,
                        lhsT=xT[:, k, ds(m16 * 128, 128)],
                        rhs=wv_sb[:, k, :],
                        start=(k == 0),
                        stop=(k == 7),
                    )
                nc.vector.tensor_tensor(
                    out=v_sb[:, m16, :, 0:64],
                    in0=vps.rearrange("p (h e) -> p h e", h=HPC),
                    in1=bv_bc.rearrange("p (h e) -> p h e", h=HPC),
                    op=Alu.add,
                )

        # ---------------- phase 3: attention ------------------------------
        with ExitStack() as ph3:
            mask_pool = ph3.enter_context(tc.tile_pool(name="maskp", bufs=1))
            expm_pool = ph3.enter_context(tc.tile_pool(name="expm", bufs=3))
            sc_ps_pool = ph3.enter_context(
                tc.tile_pool(name="scps", bufs=1, space="PSUM")
            )
            ctx_ps_pool = ph3.enter_context(
                tc.tile_pool(name="ctxps", bufs=2, space="PSUM")
            )
            rc_pool = ph3.enter_context(tc.tile_pool(name="rcp", bufs=1))

            maskf = []
            for hf in range(NHALF):
                mk = mask_pool.tile([128, SK_CHUNKS, 1024], f16, name=f"maskf{hf}")
                maskf.append(mk)
                for j in range(SK_CHUNKS):
                    nc.gpsimd.dma_start(
                        out=mk[:, j, :],
                        in_=mask_d[ds(j * 128, 128), ds(hf * 1024, 1024)],
                    )

            for hf in range(NHALF):
                # head h's denominator lives at partition 32*(h//2), slot h%2
                den_sb = rc_pool.tile([128, 2, 1024], f32, tag="den", name=f"den{hf}")
                nc.vector.memset(den_sb, 1.0)
                ctxU = rc_pool.tile([128, NPAIR, 1024], f16, tag="ctxU", name=f"ctxU{hf}")
                for c in range(NPAIR):
                    ctx_ps = [
                        ctx_ps_pool.tile([128, 1024], f32, tag="ctxps", name=f"ctxps{a}")
                        for a in range(2)
                    ]
                    # software pipeline: AV(j) is emitted after QK(j+1) so the
                    # in-order PE queue never stalls on exp/mask of iter j
                    pending = None

                    def emit_qk(j):
                        sc = sc_ps_pool.tile(
                            [128, 2, 1024], f32, tag="scps", name=f"scps{j % 2}"
                        )
                        # interleave a=0/a=1: adjacent MMs hit disjoint PE row
                        # groups (base partitions 0 / 64) and run concurrently
                        for n2 in range(2):
                            for a in range(2):
                                nc.tensor.matmul(
                                    sc[:, a, ds(n2 * 512, 512)],
                                    lhsT=kT_sb[ds(a * 64, 64), c, ds(j * 128, 128)],
                                    rhs=qT_sb[
                                        ds(a * 64, 64),
                                        c,
                                        ds(hf * 1024 + n2 * 512, 512),
                                    ],
                                    start=True,
                                    stop=True,
                                )
                        return sc

                    def emit_mask_av(j, sc):
                        # one big exp + one big mask multiply for both heads
                        expm = expm_pool.tile([128, 2, 1024], f16, tag="expm")
                        nc.scalar.activation(
                            out=expm,
                            in_=sc,
                            func=Act.Exp,
                            scale=1.0 / math.sqrt(DH),
                            bias=exp_bias,
                        )
                        expm2 = expm_pool.tile([128, 2, 1024], f16, tag="expm2")
                        nc.vector.tensor_tensor(
                            out=expm2,
                            in0=expm,
                            in1=maskf[hf][:, j, None, :].to_broadcast((128, 2, 1024)),
                            op=Alu.mult,
                        )
                        for a in range(2):
                            for n2 in range(2):
                                nc.tensor.matmul(
                                    ctx_ps[a][0:65, ds(n2 * 512, 512)],
                                    lhsT=v_sb[:, j, c * 2 + a, 0:65],
                                    rhs=expm2[:, a, ds(n2 * 512, 512)],
                                    start=(j == 0),
                                    stop=(j == SK_CHUNKS - 1),
                                )

                    for j in range(SK_CHUNKS):
                        scs = emit_qk(j)
                        if pending is not None:
                            emit_mask_av(*pending)
                        pending = (j, scs)
                    emit_mask_av(*pending)
                    for a in range(2):
                        # denom row -> batch tile (ACT), unnormalized ctx -> fp16
                        h = c * 2 + a
                        nc.scalar.copy(
                            out=den_sb[ds(32 * (h // 2), 1), h % 2, :],
                            in_=ctx_ps[a][64:65, :],
                        )
                        nc.vector.tensor_copy(
                            out=ctxU[ds(a * 64, 64), c, :], in_=ctx_ps[a][0:64, :]
                        )
                # batched reciprocal of all 8 denominators, then normalize.
                # This chain runs entirely on DVE/GPSIMD -- the PE rolls
                # straight into the next half's QK matmuls.
                den_rec = rc_pool.tile([128, 2, 1024], f32, tag="denr", name=f"denr{hf}")
                nc.vector.reciprocal(den_rec, den_sb)
                den_rec16 = rc_pool.tile(
                    [128, 2, 1024], f16, tag="denr16", name=f"denr16{hf}"
                )
                nc.vector.tensor_copy(out=den_rec16, in_=den_rec)
                # restage the 4 row-strips onto partition 0 (partition_broadcast
                # mis-reads nonzero base partitions on HW)
                deng = rc_pool.tile([1, 4, 2, 1024], f16, tag="deng", name=f"deng{hf}")
                for g in range(4):
                    nc.vector.tensor_copy(
                        out=deng[0:1, g, :, :], in_=den_rec16[ds(32 * g, 1), :, :]
                    )
                for c in range(NPAIR):
                    for a in range(2):
                        h = c * 2 + a
                        rbc = rc_pool.tile(
                            [128, 1024], f16, tag="rbc", name=f"rbc{c}_{a}", bufs=2
                        )
                        nc.gpsimd.partition_broadcast(rbc, deng[0:1, h // 2, h % 2, :])
                        nc.vector.tensor_tensor(
                            out=ctxT_sb[ds(a * 64, 64), c, ds(hf * 1024, 1024)],
                            in0=ctxU[ds(a * 64, 64), c, :],
                            in1=rbc[ds(a * 64, 64), :],
                            op=Alu.mult,
                        )

        # ---------------- phase 4: output projection -----------------------
        with ExitStack() as ph4:
            out_ps_pool = ph4.enter_context(
                tc.tile_pool(name="outps", bufs=4, space="PSUM")
            )
            ost_pool = ph4.enter_context(tc.tile_pool(name="ost", bufs=3))
            ph4_const = ph4.enter_context(tc.tile_pool(name="ph4c", bufs=1))

            # out-projection weights, cast to fp16: [128, 4, 1024]
            wo_sb = ph4_const.tile([128, 4, D], f16)
            nc.gpsimd.dma_start(
                out=wo_sb, in_=wo_d.rearrange("(r p) n -> p r n", p=128)
            )
            bo_row = ph4_const.tile([1, D], f32)
            nc.sync.dma_start(out=bo_row, in_=bo_d[None, :])
            bo_bc = ph4_const.tile([128, D], f32)
            nc.gpsimd.partition_broadcast(bo_bc, bo_row)
            for m in range(16):
                ops = out_ps_pool.tile([128, D], f32, tag="ops")
                for r in range(4):
                    for n2 in range(2):
                        nc.tensor.matmul(
                            ops[:, ds(n2 * 512, 512)],
                            lhsT=ctxT_sb[:, r, ds(m * 128, 128)],
                            rhs=wo_sb[:, r, ds(n2 * 512, 512)],
                            start=(r == 0),
                            stop=(r == 3),
                        )
                ost = ost_pool.tile([128, D], f32, tag="ost")
                nc.vector.tensor_tensor(out=ost, in0=ops, in1=bo_bc, op=Alu.add)
                nc.sync.dma_start(out=out_d[ds(m * 128, 128), :], in_=ost)

    nc.compile()
    return nc


_NC = None


def _get_nc():
    global _NC
    if _NC is None:
        _NC = _build()
    return _NC


def make_in_maps(inputs):
    x = np.ascontiguousarray(np.asarray(inputs["x"], dtype=np.float32))
    mask = np.ascontiguousarray(np.asarray(inputs["mask"], dtype=np.int32))
    w_qkv = np.asarray(inputs["w_qkv"], dtype=np.float32)
    b_qkv = np.asarray(inputs["b_qkv"], dtype=np.float32)
    w_out = np.asarray(inputs["w_out"], dtype=np.float32)
    b_out = np.asarray(inputs["b_out"], dtype=np.float32)

    in_maps = []
    for core in range(NCORES):
        b = core // 2
        h0 = (core % 2) * CD
        in_maps.append(
            {
                "x": np.ascontiguousarray(x[b]),
                # device wants mask[s_k, s_q] (scores are computed transposed);
                # DRAM holds mask[s_q, s_k] -> transpose during host-side sharding
                "mask": np.ascontiguousarray(mask[b, 0].T),
                "wq": np.ascontiguousarray(w_qkv[:, h0 : h0 + CD]),
                "wk": np.ascontiguousarray(w_qkv[:, D + h0 : D + h0 + CD]),
                "wv": np.ascontiguousarray(w_qkv[:, 2 * D + h0 : 2 * D + h0 + CD]),
                "bq": np.ascontiguousarray(b_qkv[h0 : h0 + CD]),
                "bk": np.ascontiguousarray(b_qkv[D + h0 : D + h0 + CD]),
                "bv": np.ascontiguousarray(b_qkv[2 * D + h0 : 2 * D + h0 + CD]),
                "wo": np.ascontiguousarray(w_out[h0 : h0 + CD, :]),
                "bo": b_out if core % 2 == 0 else np.zeros_like(b_out),
            }
        )
    return in_maps


def gather_out(core_outs):
    return np.stack(
        [core_outs[2 * b] + core_outs[2 * b + 1] for b in range(B)], axis=0
    )


def run(inputs, trace=False):
    """Returns (output, BassKernelResults)."""
    from concourse import bass_utils

    nc = _get_nc()
    in_maps = make_in_maps(inputs)
    res = bass_utils.run_bass_kernel_spmd(
        nc, in_maps, core_ids=list(range(NCORES)), trace=trace
    )
    out = gather_out([r["out"] for r in res.results])
    return out, res


def kernel(**inputs) -> np.ndarray:
    out, _ = run(inputs, trace=False)
    return out
